# revision 73
# baseline (speedup 1.0000x reference)
"""BertEmbedding (scalar-mix + ragged mean-pool + projection) on 8 TRN2 cores.

Full-input contract: kernel(**inputs) takes the unsharded numpy inputs and
returns the full [32, 256, 400] f32 output. Internally: data-parallel over
batch (4 examples per core), proj_w replicated (pre-transposed on host).

Structural choices (v12):
  - Ragged bound + sorted slots: positions p >= sum(bert_lens[b]) fall in
    the reference's overflow bucket and contribute nothing. Examples are
    sorted by total length and dealt round-robin to the 8 cores, so slot s
    (same on every core, SPMD) only ships/loads/processes S_s = roundup
    of the rank-group max (~[280, 266, 260, 248] vs 4x512 unsorted for the
    spec's length distribution). The host un-permutes the outputs.
  - bf16 hidden states: the rel-err tolerance (2e-2) admits bf16 for the
    bandwidth-bound hidden tensor; the host ships hid pre-cast to bf16
    (the same rounding an on-device cast would apply), halving HBM traffic
    of the dominant load. Total error stays ~4e-3.
  - Layout: positions 0..255 sit pair-interleaved (p = 2*part + q, 3KB
    bf16 runs per partition); the ragged tail (<=128 positions) sits one
    per partition in an extra q-slot so its premix/pool cost scales with
    its real size instead of the block size.
  - Equal-mix fast path: when all mix_weights entries are equal (softmax
    exactly uniform, as in the spec's fill=zeros), sum_l w_l*hid_l =
    w_0 * sum_l hid_l: three bf16 tensor-tensor adds on DVE (as a tree:
    the two pair-sums are independent), with w_0 = gamma*softmax[0]
    folded into the output's per-partition ACT scale together with 1/cnt
    (both f32). General-weights fallback premixes with per-layer scalar
    weights (tensor_scalar + 3 scalar_tensor_tensor passes).
  - Pooling matmul (bf16): pooledT[h, j] += mix_k^T @ M_k with the exact
    0/1 membership M as rhs and mixed chunks as PE weights, accumulating
    position chunks into per-h-slice PSUM banks. Tail positions (>= 256)
    only reach words j >= ceil(257/Lmax)-1 (Lmax = max lens): half width.
    Membership is built ONCE for all slots per chunk (one op-pair each):
    rows beyond a slot's bound auto-zero because ends[j] < cs there.
  - Pipeline discipline (engine streams are in-order, emission order IS
    the schedule): the four starts/ends broadcasts use four distinct PSUM
    banks and one wide SBUF tile; softmax defers behind them (eq path);
    projT's bf16 cast sits after slot0's PSUM copies on the Scalar
    stream; projection of slot s-1 is emitted after pooling of slot s so
    PE never waits on PSUM->SBUF copies; the LAST slot's copies split
    across Scalar and Vector to shorten the tail.

Membership build, scans, softmax and all scales run in exact f32.

Input-spec property relied on (declared in the problem spec):
  - bert_mask fill=ones -> positions' mask cumsum is the position index.
"""

import numpy as np

NL, B, SW, H = 4, 32, 512, 768
SL, NOUT = 256, 400
NCORES = 8
BPC = B // NCORES  # examples (slots) per core
HC = H // 128      # hidden chunks
JC = SL // 128     # word chunks

_NC_CACHE = {}
LAST_RESULT = None  # BassKernelResults of the last run (for profiling)


def _build_nc(slot_sizes, eq, jtail):
    """Per-core program: per-slot padded position bounds (even), equal-
    weights flag, word-band lower bound for tail positions (>= 256)."""
    import concourse.bacc as bacc
    import concourse.tile as tile
    from concourse import mybir

    f32 = mybir.dt.float32
    f32r = mybir.dt.float32r
    bf16 = mybir.dt.bfloat16
    i32 = mybir.dt.int32
    Alu = mybir.AluOpType
    Act = mybir.ActivationFunctionType
    Axis = mybir.AxisListType

    assert len(slot_sizes) == BPC
    geo = []       # per-slot (offset, B0, P0, R, PT, TQ, QT)
    off = 0
    maxQT = 2
    for S in slot_sizes:
        assert S % 2 == 0 and 0 < S <= SW
        B0 = min(S, 256)
        R = S - B0
        if R == 0:
            QT, PT, TQ = 2, 0, 0
        elif R <= 128:
            QT, PT, TQ = 3, R, 1
        else:
            QT, PT, TQ = 4, R // 2, 2
        geo.append((off, B0, B0 // 2, R, PT, TQ, QT))
        maxQT = max(maxQT, QT)
        off += S
    TSUM = off

    nc = bacc.Bacc(None)
    hid = nc.dram_tensor("hid", [NL, TSUM, H], bf16, kind="ExternalInput")
    lens = nc.dram_tensor("lens", [BPC, SL], i32, kind="ExternalInput")
    mw = nc.dram_tensor("mw", [1, NL], f32, kind="ExternalInput")
    gam = nc.dram_tensor("gam", [1, 1], f32, kind="ExternalInput")
    projT = nc.dram_tensor("projT", [H, NOUT], bf16, kind="ExternalInput")
    sel = nc.dram_tensor("sel", [BPC, BPC * 128], f32r, kind="ExternalInput")
    out = nc.dram_tensor("out", [BPC, SL, NOUT], f32, kind="ExternalOutput")

    with tile.TileContext(nc) as tc:
        with (
            tc.tile_pool(name="const", bufs=1) as const,
            tc.tile_pool(name="small", bufs=1) as small,
            tc.tile_pool(name="h", bufs=1) as hpool,
            tc.tile_pool(name="acc", bufs=4) as accpool,
            tc.tile_pool(name="mtmp", bufs=4) as mpool,
            tc.tile_pool(name="Mm", bufs=1) as Mpool,
            tc.tile_pool(name="se", bufs=1) as sepool,
            tc.tile_pool(name="pt", bufs=4) as ptpool,
            tc.tile_pool(name="osb", bufs=3) as opool,
            tc.tile_pool(name="psp", bufs=1, space="PSUM") as ps_p,
            tc.tile_pool(name="pso", bufs=2, space="PSUM") as ps_o,
        ):
            # ---- constants ----
            ones_f1 = const.tile([1, 128], f32)
            nc.vector.memset(ones_f1[:], 1.0)
            # dummy first ACT op: pulls ACT_TABLE_LOAD into the idle head so
            # it never delays the starts/ends PSUM copies
            actwarm = const.tile([1, 1], f32)
            nc.scalar.copy(actwarm[:], ones_f1[0:1, 0:1])
            # one-hot selector (host constant): sel[q, b*128+m] = (q == b);
            # sel_b.T @ rows[BPC, N] broadcasts rows[b] across 128 partitions
            sel_sb = const.tile([BPC, BPC * 128], f32r)
            nc.sync.dma_start(sel_sb[:], sel[:])

            # ---- lens rows first: they gate the ends/starts scan ----
            lens_i = small.tile([BPC, SL], i32)
            nc.sync.dma_start(lens_i[:], lens[:])
            # inverse-count loads (tiny, HWDGE, land in a few us)
            lensc_i = small.tile([128, JC, BPC], i32)
            for jh in range(JC):
                nc.sync.dma_start(lensc_i[:, jh, :], lens[:, jh * 128:(jh + 1) * 128].rearrange("b p -> p b"))
            mw_sb = small.tile([1, NL], f32)
            nc.sync.dma_start(mw_sb[:], mw[:])
            gam_sb = small.tile([1, 1], f32)
            nc.sync.dma_start(gam_sb[:], gam[:])

            # ---- per-position ids (gpsimd stream head, before descgens) ----
            cs_i = small.tile([128, maxQT], i32)
            nc.gpsimd.iota(cs_i[:, 0:2], pattern=[[1, 2]], base=1, channel_multiplier=2)
            if maxQT == 3:
                nc.gpsimd.iota(cs_i[:, 2:3], pattern=[[1, 1]], base=257, channel_multiplier=1)
            elif maxQT == 4:
                nc.gpsimd.iota(cs_i[:, 2:4], pattern=[[1, 2]], base=257, channel_multiplier=2)
            cs_sb = small.tile([128, maxQT], f32)
            nc.vector.tensor_copy(cs_sb[:], cs_i[:])

            # ---- lens: ends/starts rows (f32r) ----
            lensf = small.tile([BPC, SL], f32)
            nc.vector.tensor_copy(lensf[:], lens_i[:])
            ends_r = small.tile([BPC, SL], f32r)
            nc.vector.tensor_tensor_scan(out=ends_r[:], data0=lensf[:], data1=lensf[:], initial=0.0, op0=Alu.add, op1=Alu.bypass)
            starts_r = small.tile([BPC, SL], f32r)
            nc.vector.tensor_sub(starts_r[:], ends_r[:], lensf[:])

            def emit_softmax():
                # softmax(mix_weights) * gamma, broadcast to [128, NL]
                mmax = small.tile([1, 1], f32)
                nc.vector.tensor_reduce(out=mmax[:], in_=mw_sb[:], axis=Axis.X, op=Alu.max)
                nmax = small.tile([1, 1], f32)
                nc.vector.tensor_scalar(out=nmax[:], in0=mmax[:], scalar1=-1.0, scalar2=None, op0=Alu.mult)
                mexp = small.tile([1, NL], f32)
                nc.scalar.activation(out=mexp[:], in_=mw_sb[:], func=Act.Exp, bias=nmax[:], scale=1.0)
                msum = small.tile([1, 1], f32)
                nc.vector.tensor_reduce(out=msum[:], in_=mexp[:], axis=Axis.X, op=Alu.add)
                mrec = small.tile([1, 1], f32)
                nc.vector.reciprocal(out=mrec[:], in_=msum[:])
                w_row = small.tile([1, NL], f32)
                nc.vector.tensor_scalar(out=w_row[:], in0=mexp[:], scalar1=mrec[:], scalar2=gam_sb[:], op0=Alu.mult, op1=Alu.mult)
                ps_w = ps_o.tile([128, NL], f32, tag="po")
                nc.tensor.matmul(out=ps_w[:], lhsT=ones_f1[:], rhs=w_row[:], start=True, stop=True)
                w_sb = small.tile([128, NL], f32)
                nc.scalar.copy(w_sb[:], ps_w[:])
                return w_sb

            # general path needs the weights before the first premix; the
            # eq path only needs w_0 for the output scale, so its softmax
            # is deferred off the head-critical Vector stream
            w_sb = None if eq else emit_softmax()

            # ---- starts/ends broadcasts: four distinct PSUM banks so they
            # don't serialize through copies; one wide SBUF tile so the
            # membership build runs as one op per chunk for all slots
            se_all = sepool.tile([128, BPC, 2 * SL], f32)
            for b in range(BPC):
                ps_se = ps_p.tile([128, 2 * SL], f32, tag=f"pp{b}", name=f"se{b}")
                sel_b = sel_sb[:, b * 128:(b + 1) * 128]
                nc.tensor.matmul(out=ps_se[:, 0:SL], lhsT=sel_b, rhs=starts_r[:], start=True, stop=True)
                nc.tensor.matmul(out=ps_se[:, SL:2 * SL], lhsT=sel_b, rhs=ends_r[:], start=True, stop=True)
                nc.scalar.copy(se_all[:, b, :], ps_se[:])

            # ---- hidden loads: bulk + ragged tail per (slot, layer) ----
            hts_all = [[] for _ in range(BPC)]

            def emit_hid(b, l):
                off, B0, P0, R, PT, TQ, QT = geo[b]
                ht = hpool.tile([128, QT, H], bf16, tag=f"h{b}_{l}", name=f"h{b}_{l}")
                nc.gpsimd.dma_start(
                    ht[0:P0, 0:2, :],
                    hid[l, off:off + B0, :].rearrange("(p q) d -> p q d", q=2))
                if TQ == 1:
                    nc.gpsimd.dma_start(ht[0:PT, 2, :], hid[l, off + B0:off + B0 + R, :])
                elif TQ == 2:
                    nc.gpsimd.dma_start(
                        ht[0:PT, 2:4, :],
                        hid[l, off + B0:off + B0 + R, :].rearrange("(p q) d -> p q d", q=2))
                hts_all[b].append(ht)

            for b in range(BPC):
                for l in range(NL):
                    emit_hid(b, l)
                if b == 0:
                    # deferred: projT load behind the first slot's hid descgen
                    projT_sb = const.tile([128, HC, NOUT], bf16)
                    nc.gpsimd.dma_start(projT_sb[:], projT.rearrange("(i p) o -> p i o", p=128))

            if eq:
                w_sb = emit_softmax()

            # ---- inverse counts + combined output scale (f32, exact) ----
            lensc_f = small.tile([128, JC, BPC], f32)
            nc.vector.tensor_copy(lensc_f[:], lensc_i[:])
            lensc_m = small.tile([128, JC, BPC], f32)
            nc.vector.tensor_scalar_max(lensc_m[:], lensc_f[:], 1.0)
            invcnt = small.tile([128, JC, BPC], f32)
            nc.vector.reciprocal(out=invcnt[:], in_=lensc_m[:])
            osc = small.tile([128, JC, BPC], f32)
            if eq:
                nc.vector.tensor_scalar(out=osc[:], in0=invcnt[:], scalar1=w_sb[:, 0:1], scalar2=None, op0=Alu.mult)
            else:
                nc.vector.tensor_copy(osc[:], invcnt[:])

            def proj_act_out(b, ptsb):
                # projection (bf16) + combined scale on the PSUM->SBUF copy
                osb = opool.tile([128, JC, NOUT], f32, tag="o")
                for jh in range(JC):
                    po = ps_o.tile([128, NOUT], f32, tag="po")
                    for i in range(HC):
                        nc.tensor.matmul(
                            out=po[:],
                            lhsT=ptsb[:, i, jh * 128:(jh + 1) * 128],
                            rhs=projT_sb[:, i, :],
                            start=(i == 0),
                            stop=(i == HC - 1),
                        )
                    nc.scalar.activation(out=osb[:, jh, :], in_=po[:], func=Act.Copy, scale=osc[:, jh, b:b + 1])
                    nc.scalar.dma_start(out[b, jh * 128:(jh + 1) * 128, :], osb[:, jh, :])

            # ---- per-slot pipeline: premix + membership + pool, with the
            # projection of the previous slot emitted after this pool ----
            pending = None
            Mt_all = Mpool.tile([128, maxQT, BPC, SL], bf16, name="Mall")
            for b in range(BPC):
                off, B0, P0, R, PT, TQ, QT = geo[b]
                chunks = [(q, P0, 0) for q in range(2)] + [(2 + t, PT, jtail) for t in range(TQ)]
                hts = hts_all[b]
                regions = [(slice(0, P0), slice(0, 2))]
                if TQ:
                    regions.append((slice(0, PT), slice(2, QT)))
                mm = accpool.tile([128, QT, H], bf16, tag="mm", name="mm")
                qsplit = eq
                mmq = None
                if qsplit:
                    # per-q mixed tiles so pooling on chunk q0 starts
                    # while q1 (and the tail) are still premixing
                    mmq = [accpool.tile([128, 1, H], bf16, tag=f"mq{q}", name=f"mq{q}")
                           for q in range(2)]
                if eq:
                    # unweighted layer sum (scale folded into output) as a
                    # tree: the two pair-sums are independent, so the
                    # sequencer dispatches them back-to-back
                    s01 = accpool.tile([128, QT, H], bf16, tag="s01")
                    a2 = accpool.tile([128, QT, H], bf16, tag="a2")
                    for ri, (ps, qs) in enumerate(regions):
                        nc.vector.tensor_add(s01[ps, qs], hts[0][ps, qs], hts[1][ps, qs])
                        nc.vector.tensor_add(a2[ps, qs], hts[2][ps, qs], hts[3][ps, qs])
                        if qsplit and ri == 0:
                            for q in range(2):
                                nc.vector.tensor_add(
                                    mmq[q][ps, 0:1, :], s01[ps, q:q + 1, :], a2[ps, q:q + 1, :])
                        else:
                            nc.vector.tensor_add(mm[ps, qs], s01[ps, qs], a2[ps, qs])
                else:
                    # premix: mixed = sum_l w[l] * hid[l] (DVE)
                    s01 = accpool.tile([128, QT, H], bf16, tag="s01")
                    a2 = accpool.tile([128, QT, H], bf16, tag="a2")
                    for ps, qs in regions:
                        prev = None
                        for l in range(NL):
                            dst = mm if l == NL - 1 else (s01 if l == 0 else a2)
                            wl = w_sb[ps, l:l + 1]
                            if l == 0:
                                nc.vector.tensor_scalar(
                                    out=dst[ps, qs], in0=hts[l][ps, qs],
                                    scalar1=wl, scalar2=None, op0=Alu.mult)
                            else:
                                nc.vector.scalar_tensor_tensor(
                                    out=dst[ps, qs], in0=hts[l][ps, qs],
                                    scalar=wl, in1=prev[ps, qs], op0=Alu.mult, op1=Alu.add)
                            prev = dst

                if b == 0:
                    # membership for ALL slots, one op-pair per chunk
                    # (M = exact 0/1, bf16); rows beyond a slot's bound
                    # auto-zero because ends[j] < cs there. Emitted after
                    # slot0's premix so the ready-to-run adds never queue
                    # behind M's sem waits.
                    mchunks = [(0, 0), (1, 0)] + ([(2 + t, jtail) for t in range(maxQT - 2)])
                    for k, j0 in mchunks:
                        csc = cs_sb[:, k:k + 1]
                        m2 = mpool.tile([128, BPC, SL], f32, tag="m2")
                        nc.vector.tensor_scalar(
                            out=m2[:, :, j0:], in0=se_all[:, :, SL + j0:2 * SL], scalar1=csc,
                            scalar2=None, op0=Alu.is_ge)
                        nc.vector.scalar_tensor_tensor(
                            out=Mt_all[:, k, :, j0:], in0=se_all[:, :, j0:SL], scalar=csc,
                            in1=m2[:, :, j0:], op0=Alu.is_lt, op1=Alu.mult)

                # ---- ragged mean-pool: pooledT[h, j] += mix_k^T @ M_k ----
                # one PSUM bank per h-slice: interleaved accumulation groups
                # are only correct across different banks (HW-verified)
                pps = []
                for i in range(HC):
                    pp_i = ps_p.tile([128, SL], f32, tag=f"pp{i}", name=f"pp{i}")
                    pps.append(pp_i)
                nk = len(chunks)
                for ci, (k, pc, j0) in enumerate(chunks):
                    for i in range(HC):
                        lhsT_src = mmq[k][0:pc, 0, i * 128:(i + 1) * 128] if (mmq is not None and k < 2) \
                            else mm[0:pc, k, i * 128:(i + 1) * 128]
                        nc.tensor.matmul(
                            out=pps[i][:, j0:],
                            lhsT=lhsT_src,
                            rhs=Mt_all[0:pc, k, b, j0:],
                            start=(ci == 0),
                            stop=(ci == nk - 1),
                            skip_group_check=True,
                        )
                ptsb = ptpool.tile([128, HC, SL], bf16, tag="pt")
                for i in range(HC):
                    if b == BPC - 1 and i % 2 == 1:
                        # last slot: split copies across engines (tail)
                        nc.vector.tensor_copy(ptsb[:, i, :], pps[i][:])
                    else:
                        nc.scalar.copy(ptsb[:, i, :], pps[i][:])
                if pending is not None:
                    proj_act_out(*pending)
                pending = (b, ptsb)
            proj_act_out(*pending)

    nc.finalize()
    return nc


def _get_nc(key):
    if key not in _NC_CACHE:
        _NC_CACHE[key] = _build_nc(*key)
    return _NC_CACHE[key]


def kernel(subwords=None, bert_lens=None, bert_mask=None, hidden_states=None,
           mix_weights=None, gamma=None, proj_w=None, **_ignored):
    global LAST_RESULT
    import os
    import ml_dtypes
    from concourse.bass_utils import run_bass_kernel_spmd

    hs = np.asarray(hidden_states, dtype=np.float32)
    lens_np = np.asarray(bert_lens).astype(np.int32)
    mw_np = np.asarray(mix_weights, dtype=np.float32).reshape(1, NL)
    gam_np = np.asarray(gamma, dtype=np.float32).reshape(1, 1)
    projT_np = np.ascontiguousarray(np.asarray(proj_w, dtype=np.float32).T).astype(ml_dtypes.bfloat16)
    sel_np = np.zeros((BPC, BPC * 128), dtype=np.float32)
    for b in range(BPC):
        sel_np[b, b * 128:(b + 1) * 128] = 1.0

    # program specialization from the runtime inputs (cached per key):
    # sorted-slot ragged bounds, equal-weights path, tail word band
    Tb = lens_np.sum(axis=1).astype(np.int64)
    order = np.argsort(-Tb, kind="stable")  # example -> (slot, core)
    slot_sizes = []
    for s in range(BPC):
        grp = order[s * NCORES:(s + 1) * NCORES]
        S = int(min(max(int(Tb[grp].max()), 2), SW))
        S += S % 2
        slot_sizes.append(S)
    slot_sizes = tuple(slot_sizes)
    eq = bool(np.all(mw_np == mw_np.flat[0]))
    Lmax = max(int(lens_np.max()), 1)
    jtail = int(max(0, min(SL - 1, -(-257 // Lmax) - 1)))
    nc = _get_nc((slot_sizes, eq, jtail))

    hs_b = hs.astype(ml_dtypes.bfloat16)
    in_maps = []
    for c in range(NCORES):
        exs = [int(order[s * NCORES + c]) for s in range(BPC)]
        hid_c = np.concatenate(
            [hs_b[:, e, :slot_sizes[s], :] for s, e in enumerate(exs)], axis=1)
        in_maps.append({
            "hid": np.ascontiguousarray(hid_c),
            "lens": np.ascontiguousarray(lens_np[exs]),
            "mw": mw_np,
            "gam": gam_np,
            "projT": projT_np,
            "sel": sel_np,
        })

    trace = bool(int(os.environ.get("KERNEL_TRACE", "0")))
    LAST_RESULT = run_bass_kernel_spmd(nc, in_maps, list(range(NCORES)), trace=trace)
    res = LAST_RESULT.results

    full = np.empty((B, SL, NOUT), dtype=np.float32)
    for c in range(NCORES):
        o = res[c]["out"]
        for s in range(BPC):
            full[int(order[s * NCORES + c])] = o[s]
    return full


# revision 74
# speedup vs baseline: 1.0739x; 1.0739x over previous
"""BertEmbedding (scalar-mix + ragged mean-pool + projection) on 8 TRN2 cores.

Full-input contract: kernel(**inputs) takes the unsharded numpy inputs and
returns the full [32, 256, 400] f32 output. Internally: data-parallel over
batch (4 examples per core), proj_w replicated (pre-transposed on host).

Structural choices (v12):
  - Ragged bound + sorted slots: positions p >= sum(bert_lens[b]) fall in
    the reference's overflow bucket and contribute nothing. Examples are
    sorted by total length and dealt round-robin to the 8 cores, so slot s
    (same on every core, SPMD) only ships/loads/processes S_s = roundup
    of the rank-group max (~[280, 266, 260, 248] vs 4x512 unsorted for the
    spec's length distribution). The host un-permutes the outputs.
  - bf16 hidden states: the rel-err tolerance (2e-2) admits bf16 for the
    bandwidth-bound hidden tensor; the host ships hid pre-cast to bf16
    (the same rounding an on-device cast would apply), halving HBM traffic
    of the dominant load. Total error stays ~4e-3.
  - Layout: positions 0..255 sit pair-interleaved (p = 2*part + q, 3KB
    bf16 runs per partition); the ragged tail (<=128 positions) sits one
    per partition in an extra q-slot so its premix/pool cost scales with
    its real size instead of the block size.
  - Equal-mix fast path: when all mix_weights entries are equal (softmax
    exactly uniform, as in the spec's fill=zeros), sum_l w_l*hid_l =
    w_0 * sum_l hid_l: three bf16 tensor-tensor adds on DVE (as a tree:
    the two pair-sums are independent), with w_0 = gamma*softmax[0]
    folded into the output's per-partition ACT scale together with 1/cnt
    (both f32). General-weights fallback premixes with per-layer scalar
    weights (tensor_scalar + 3 scalar_tensor_tensor passes).
  - Pooling matmul (bf16): pooledT[h, j] += mix_k^T @ M_k with the exact
    0/1 membership M as rhs and mixed chunks as PE weights, accumulating
    position chunks into per-h-slice PSUM banks. Tail positions (>= 256)
    only reach words j >= ceil(257/Lmax)-1 (Lmax = max lens): half width.
    Membership is built ONCE for all slots per chunk (one op-pair each):
    rows beyond a slot's bound auto-zero because ends[j] < cs there.
  - Pipeline discipline (engine streams are in-order, emission order IS
    the schedule): the four starts/ends broadcasts use four distinct PSUM
    banks and one wide SBUF tile; softmax defers behind them (eq path);
    projT's bf16 cast sits after slot0's PSUM copies on the Scalar
    stream; projection of slot s-1 is emitted after pooling of slot s so
    PE never waits on PSUM->SBUF copies; the LAST slot's copies split
    across Scalar and Vector to shorten the tail.

Membership build, scans, softmax and all scales run in exact f32.

Input-spec property relied on (declared in the problem spec):
  - bert_mask fill=ones -> positions' mask cumsum is the position index.
"""

import numpy as np

NL, B, SW, H = 4, 32, 512, 768
SL, NOUT = 256, 400
NCORES = 8
BPC = B // NCORES  # examples (slots) per core
HC = H // 128      # hidden chunks
JC = SL // 128     # word chunks

_NC_CACHE = {}
LAST_RESULT = None  # BassKernelResults of the last run (for profiling)


def _build_nc(slot_sizes, eq, jtail):
    """Per-core program: per-slot padded position bounds (even), equal-
    weights flag, word-band lower bound for tail positions (>= 256)."""
    import concourse.bacc as bacc
    import concourse.tile as tile
    from concourse import mybir

    f32 = mybir.dt.float32
    f32r = mybir.dt.float32r
    bf16 = mybir.dt.bfloat16
    i32 = mybir.dt.int32
    Alu = mybir.AluOpType
    Act = mybir.ActivationFunctionType
    Axis = mybir.AxisListType

    assert len(slot_sizes) == BPC
    geo = []       # per-slot (offset, B0, P0, R, PT, TQ, QT)
    off = 0
    maxQT = 2
    for S in slot_sizes:
        assert S % 2 == 0 and 0 < S <= SW
        B0 = min(S, 256)
        R = S - B0
        if R == 0:
            QT, PT, TQ = 2, 0, 0
        elif R <= 128:
            QT, PT, TQ = 3, R, 1
        else:
            QT, PT, TQ = 4, R // 2, 2
        geo.append((off, B0, B0 // 2, R, PT, TQ, QT))
        maxQT = max(maxQT, QT)
        off += S
    TSUM = off

    nc = bacc.Bacc(None)
    hid = nc.dram_tensor("hid", [NL, TSUM, H], bf16, kind="ExternalInput")
    lens = nc.dram_tensor("lens", [BPC, SL], i32, kind="ExternalInput")
    mw = nc.dram_tensor("mw", [1, NL], f32, kind="ExternalInput")
    gam = nc.dram_tensor("gam", [1, 1], f32, kind="ExternalInput")
    projT = nc.dram_tensor("projT", [H, NOUT], bf16, kind="ExternalInput")
    sel = nc.dram_tensor("sel", [BPC, BPC * 128], f32r, kind="ExternalInput")
    out = nc.dram_tensor("out", [BPC, SL, NOUT], f32, kind="ExternalOutput")

    with tile.TileContext(nc) as tc:
        with (
            tc.tile_pool(name="const", bufs=1) as const,
            tc.tile_pool(name="small", bufs=1) as small,
            tc.tile_pool(name="h", bufs=1) as hpool,
            tc.tile_pool(name="acc", bufs=4) as accpool,
            tc.tile_pool(name="mtmp", bufs=4) as mpool,
            tc.tile_pool(name="Mm", bufs=1) as Mpool,
            tc.tile_pool(name="se", bufs=1) as sepool,
            tc.tile_pool(name="pt", bufs=3) as ptpool,
            tc.tile_pool(name="osb", bufs=2) as opool,
            tc.tile_pool(name="psp", bufs=1, space="PSUM") as ps_p,
            tc.tile_pool(name="pso", bufs=2, space="PSUM") as ps_o,
        ):
            # ---- constants ----
            ones_f1 = const.tile([1, 128], f32)
            nc.vector.memset(ones_f1[:], 1.0)
            # dummy first ACT op: pulls ACT_TABLE_LOAD into the idle head so
            # it never delays the starts/ends PSUM copies
            actwarm = const.tile([1, 1], f32)
            nc.scalar.copy(actwarm[:], ones_f1[0:1, 0:1])
            # one-hot selector (host constant): sel[q, b*128+m] = (q == b);
            # sel_b.T @ rows[BPC, N] broadcasts rows[b] across 128 partitions
            sel_sb = const.tile([BPC, BPC * 128], f32r)
            nc.sync.dma_start(sel_sb[:], sel[:])

            # ---- lens rows first: they gate the ends/starts scan ----
            lens_i = small.tile([BPC, SL], i32)
            nc.sync.dma_start(lens_i[:], lens[:])
            # inverse-count loads (tiny, HWDGE, land in a few us)
            lensc_i = small.tile([128, JC, BPC], i32)
            for jh in range(JC):
                nc.sync.dma_start(lensc_i[:, jh, :], lens[:, jh * 128:(jh + 1) * 128].rearrange("b p -> p b"))
            mw_sb = small.tile([1, NL], f32)
            nc.sync.dma_start(mw_sb[:], mw[:])
            gam_sb = small.tile([1, 1], f32)
            nc.sync.dma_start(gam_sb[:], gam[:])

            # ---- per-position ids (gpsimd stream head, before descgens) ----
            cs_i = small.tile([128, maxQT], i32)
            nc.gpsimd.iota(cs_i[:, 0:2], pattern=[[1, 2]], base=1, channel_multiplier=2)
            if maxQT == 3:
                nc.gpsimd.iota(cs_i[:, 2:3], pattern=[[1, 1]], base=257, channel_multiplier=1)
            elif maxQT == 4:
                nc.gpsimd.iota(cs_i[:, 2:4], pattern=[[1, 2]], base=257, channel_multiplier=2)
            cs_sb = small.tile([128, maxQT], f32)
            nc.vector.tensor_copy(cs_sb[:], cs_i[:])

            # ---- lens: ends/starts rows (f32r) ----
            lensf = small.tile([BPC, SL], f32)
            nc.vector.tensor_copy(lensf[:], lens_i[:])
            ends_r = small.tile([BPC, SL], f32r)
            nc.vector.tensor_tensor_scan(out=ends_r[:], data0=lensf[:], data1=lensf[:], initial=0.0, op0=Alu.add, op1=Alu.bypass)
            starts_r = small.tile([BPC, SL], f32r)
            nc.vector.tensor_sub(starts_r[:], ends_r[:], lensf[:])

            def emit_softmax():
                # softmax(mix_weights) * gamma, broadcast to [128, NL]
                mmax = small.tile([1, 1], f32)
                nc.vector.tensor_reduce(out=mmax[:], in_=mw_sb[:], axis=Axis.X, op=Alu.max)
                nmax = small.tile([1, 1], f32)
                nc.vector.tensor_scalar(out=nmax[:], in0=mmax[:], scalar1=-1.0, scalar2=None, op0=Alu.mult)
                mexp = small.tile([1, NL], f32)
                nc.scalar.activation(out=mexp[:], in_=mw_sb[:], func=Act.Exp, bias=nmax[:], scale=1.0)
                msum = small.tile([1, 1], f32)
                nc.vector.tensor_reduce(out=msum[:], in_=mexp[:], axis=Axis.X, op=Alu.add)
                mrec = small.tile([1, 1], f32)
                nc.vector.reciprocal(out=mrec[:], in_=msum[:])
                w_row = small.tile([1, NL], f32)
                nc.vector.tensor_scalar(out=w_row[:], in0=mexp[:], scalar1=mrec[:], scalar2=gam_sb[:], op0=Alu.mult, op1=Alu.mult)
                ps_w = ps_o.tile([128, NL], f32, tag="po")
                nc.tensor.matmul(out=ps_w[:], lhsT=ones_f1[:], rhs=w_row[:], start=True, stop=True)
                w_sb = small.tile([128, NL], f32)
                nc.scalar.copy(w_sb[:], ps_w[:])
                return w_sb

            # general path needs the weights before the first premix; the
            # eq path only needs w_0 for the output scale, so its softmax
            # is deferred off the head-critical Vector stream
            w_sb = None if eq else emit_softmax()

            # ---- starts/ends broadcasts: four distinct PSUM banks so they
            # don't serialize through copies; one wide SBUF tile so the
            # membership build runs as one op per chunk for all slots
            se_all = sepool.tile([128, BPC, 2 * SL], f32)
            for b in range(BPC):
                ps_se = ps_p.tile([128, 2 * SL], f32, tag=f"pp{b}", name=f"se{b}")
                sel_b = sel_sb[:, b * 128:(b + 1) * 128]
                nc.tensor.matmul(out=ps_se[:, 0:SL], lhsT=sel_b, rhs=starts_r[:], start=True, stop=True)
                nc.tensor.matmul(out=ps_se[:, SL:2 * SL], lhsT=sel_b, rhs=ends_r[:], start=True, stop=True)
                nc.scalar.copy(se_all[:, b, :], ps_se[:])

            # ---- hidden loads: bulk + ragged tail per (slot, layer) ----
            hts_all = [[] for _ in range(BPC)]

            def emit_hid(b, l):
                off, B0, P0, R, PT, TQ, QT = geo[b]
                ht = hpool.tile([128, QT, H], bf16, tag=f"h{b}_{l}", name=f"h{b}_{l}")
                nc.gpsimd.dma_start(
                    ht[0:P0, 0:2, :],
                    hid[l, off:off + B0, :].rearrange("(p q) d -> p q d", q=2))
                if TQ == 1:
                    nc.gpsimd.dma_start(ht[0:PT, 2, :], hid[l, off + B0:off + B0 + R, :])
                elif TQ == 2:
                    nc.gpsimd.dma_start(
                        ht[0:PT, 2:4, :],
                        hid[l, off + B0:off + B0 + R, :].rearrange("(p q) d -> p q d", q=2))
                hts_all[b].append(ht)

            for b in range(BPC):
                for l in range(NL):
                    emit_hid(b, l)
                if b == 0:
                    # deferred: projT load behind the first slot's hid descgen
                    projT_sb = const.tile([128, HC, NOUT], bf16)
                    nc.gpsimd.dma_start(projT_sb[:], projT.rearrange("(i p) o -> p i o", p=128))

            if eq:
                w_sb = emit_softmax()

            # ---- inverse counts + combined output scale (f32, exact) ----
            lensc_f = small.tile([128, JC, BPC], f32)
            nc.vector.tensor_copy(lensc_f[:], lensc_i[:])
            lensc_m = small.tile([128, JC, BPC], f32)
            nc.vector.tensor_scalar_max(lensc_m[:], lensc_f[:], 1.0)
            invcnt = small.tile([128, JC, BPC], f32)
            nc.vector.reciprocal(out=invcnt[:], in_=lensc_m[:])
            osc = small.tile([128, JC, BPC], f32)
            if eq:
                nc.vector.tensor_scalar(out=osc[:], in0=invcnt[:], scalar1=w_sb[:, 0:1], scalar2=None, op0=Alu.mult)
            else:
                nc.vector.tensor_copy(osc[:], invcnt[:])

            def proj_act_out(b, ptsb):
                # projection (bf16) + combined scale on the PSUM->SBUF copy
                osb = opool.tile([128, JC, NOUT], f32, tag="o")
                for jh in range(JC):
                    po = ps_o.tile([128, NOUT], f32, tag="po")
                    for i in range(HC):
                        nc.tensor.matmul(
                            out=po[:],
                            lhsT=ptsb[:, i, jh * 128:(jh + 1) * 128],
                            rhs=projT_sb[:, i, :],
                            start=(i == 0),
                            stop=(i == HC - 1),
                        )
                    nc.scalar.activation(out=osb[:, jh, :], in_=po[:], func=Act.Copy, scale=osc[:, jh, b:b + 1])
                    nc.scalar.dma_start(out[b, jh * 128:(jh + 1) * 128, :], osb[:, jh, :])

            # ---- per-slot pipeline: premix + membership + pool, with the
            # projection of the previous slot emitted after this pool ----
            pending = None
            Mt_all = Mpool.tile([128, maxQT, BPC, SL], bf16, name="Mall")
            for b in range(BPC):
                off, B0, P0, R, PT, TQ, QT = geo[b]
                chunks = [(q, P0, 0) for q in range(2)] + [(2 + t, PT, jtail) for t in range(TQ)]
                hts = hts_all[b]
                regions = [(slice(0, P0), slice(0, 2))]
                if TQ:
                    regions.append((slice(0, PT), slice(2, QT)))
                mm = accpool.tile([128, QT, H], bf16, tag="mm", name="mm")
                qsplit = eq
                mmq = None
                if qsplit:
                    # per-q mixed tiles so pooling on chunk q0 starts
                    # while q1 (and the tail) are still premixing
                    mmq = [accpool.tile([128, 1, H], bf16, tag=f"mq{q}", name=f"mq{q}")
                           for q in range(2)]
                if eq:
                    # unweighted layer sum (scale folded into output) as a
                    # tree: the two pair-sums are independent, so the
                    # sequencer dispatches them back-to-back
                    s01 = accpool.tile([128, QT, H], bf16, tag="s01")
                    a2 = accpool.tile([128, QT, H], bf16, tag="a2")
                    for ri, (ps, qs) in enumerate(regions):
                        nc.vector.tensor_add(s01[ps, qs], hts[0][ps, qs], hts[1][ps, qs])
                        nc.vector.tensor_add(a2[ps, qs], hts[2][ps, qs], hts[3][ps, qs])
                        if qsplit and ri == 0:
                            for q in range(2):
                                nc.vector.tensor_add(
                                    mmq[q][ps, 0:1, :], s01[ps, q:q + 1, :], a2[ps, q:q + 1, :])
                        else:
                            nc.vector.tensor_add(mm[ps, qs], s01[ps, qs], a2[ps, qs])
                else:
                    # premix: mixed = sum_l w[l] * hid[l] (DVE)
                    s01 = accpool.tile([128, QT, H], bf16, tag="s01")
                    a2 = accpool.tile([128, QT, H], bf16, tag="a2")
                    for ps, qs in regions:
                        prev = None
                        for l in range(NL):
                            dst = mm if l == NL - 1 else (s01 if l == 0 else a2)
                            wl = w_sb[ps, l:l + 1]
                            if l == 0:
                                nc.vector.tensor_scalar(
                                    out=dst[ps, qs], in0=hts[l][ps, qs],
                                    scalar1=wl, scalar2=None, op0=Alu.mult)
                            else:
                                nc.vector.scalar_tensor_tensor(
                                    out=dst[ps, qs], in0=hts[l][ps, qs],
                                    scalar=wl, in1=prev[ps, qs], op0=Alu.mult, op1=Alu.add)
                            prev = dst

                if b == 0:
                    # membership for ALL slots, one op-pair per chunk
                    # (M = exact 0/1, bf16); rows beyond a slot's bound
                    # auto-zero because ends[j] < cs there. Emitted after
                    # slot0's premix so the ready-to-run adds never queue
                    # behind M's sem waits.
                    mchunks = [(0, 0), (1, 0)] + ([(2 + t, jtail) for t in range(maxQT - 2)])
                    for k, j0 in mchunks:
                        csc = cs_sb[:, k:k + 1]
                        m2 = mpool.tile([128, BPC, SL], f32, tag="m2")
                        nc.vector.tensor_scalar(
                            out=m2[:, :, j0:], in0=se_all[:, :, SL + j0:2 * SL], scalar1=csc,
                            scalar2=None, op0=Alu.is_ge)
                        nc.vector.scalar_tensor_tensor(
                            out=Mt_all[:, k, :, j0:], in0=se_all[:, :, j0:SL], scalar=csc,
                            in1=m2[:, :, j0:], op0=Alu.is_lt, op1=Alu.mult)

                # ---- ragged mean-pool: pooledT[h, j] += mix_k^T @ M_k ----
                # one PSUM bank per h-slice: interleaved accumulation groups
                # are only correct across different banks (HW-verified)
                pps = []
                for i in range(HC):
                    pp_i = ps_p.tile([128, SL], f32, tag=f"pp{i}", name=f"pp{i}")
                    pps.append(pp_i)
                nk = len(chunks)
                for ci, (k, pc, j0) in enumerate(chunks):
                    for i in range(HC):
                        lhsT_src = mmq[k][0:pc, 0, i * 128:(i + 1) * 128] if (mmq is not None and k < 2) \
                            else mm[0:pc, k, i * 128:(i + 1) * 128]
                        nc.tensor.matmul(
                            out=pps[i][:, j0:],
                            lhsT=lhsT_src,
                            rhs=Mt_all[0:pc, k, b, j0:],
                            start=(ci == 0),
                            stop=(ci == nk - 1),
                            skip_group_check=True,
                        )
                ptsb = ptpool.tile([128, HC, SL], bf16, tag="pt")
                for i in range(HC):
                    if b == BPC - 1 and i % 2 == 1:
                        # last slot: split copies across engines (tail)
                        nc.vector.tensor_copy(ptsb[:, i, :], pps[i][:])
                    else:
                        nc.scalar.copy(ptsb[:, i, :], pps[i][:])
                if pending is not None:
                    proj_act_out(*pending)
                pending = (b, ptsb)
            proj_act_out(*pending)

    nc.finalize()
    return nc


def _get_nc(key):
    if key not in _NC_CACHE:
        _NC_CACHE[key] = _build_nc(*key)
    return _NC_CACHE[key]


def kernel(subwords=None, bert_lens=None, bert_mask=None, hidden_states=None,
           mix_weights=None, gamma=None, proj_w=None, **_ignored):
    global LAST_RESULT
    import os
    import ml_dtypes
    from concourse.bass_utils import run_bass_kernel_spmd

    hs = np.asarray(hidden_states, dtype=np.float32)
    lens_np = np.asarray(bert_lens).astype(np.int32)
    mw_np = np.asarray(mix_weights, dtype=np.float32).reshape(1, NL)
    gam_np = np.asarray(gamma, dtype=np.float32).reshape(1, 1)
    projT_np = np.ascontiguousarray(np.asarray(proj_w, dtype=np.float32).T).astype(ml_dtypes.bfloat16)
    sel_np = np.zeros((BPC, BPC * 128), dtype=np.float32)
    for b in range(BPC):
        sel_np[b, b * 128:(b + 1) * 128] = 1.0

    # program specialization from the runtime inputs (cached per key):
    # sorted-slot ragged bounds, equal-weights path, tail word band
    Tb = lens_np.sum(axis=1).astype(np.int64)
    order = np.argsort(-Tb, kind="stable")  # example -> (slot, core)
    slot_sizes = []
    for s in range(BPC):
        grp = order[s * NCORES:(s + 1) * NCORES]
        S = int(min(max(int(Tb[grp].max()), 2), SW))
        S += S % 2
        slot_sizes.append(S)
    slot_sizes = tuple(slot_sizes)
    eq = bool(np.all(mw_np == mw_np.flat[0]))
    Lmax = max(int(lens_np.max()), 1)
    jtail = int(max(0, min(SL - 1, -(-257 // Lmax) - 1)))
    nc = _get_nc((slot_sizes, eq, jtail))

    hs_b = hs.astype(ml_dtypes.bfloat16)
    in_maps = []
    for c in range(NCORES):
        exs = [int(order[s * NCORES + c]) for s in range(BPC)]
        hid_c = np.concatenate(
            [hs_b[:, e, :slot_sizes[s], :] for s, e in enumerate(exs)], axis=1)
        in_maps.append({
            "hid": np.ascontiguousarray(hid_c),
            "lens": np.ascontiguousarray(lens_np[exs]),
            "mw": mw_np,
            "gam": gam_np,
            "projT": projT_np,
            "sel": sel_np,
        })

    trace = bool(int(os.environ.get("KERNEL_TRACE", "0")))
    LAST_RESULT = run_bass_kernel_spmd(nc, in_maps, list(range(NCORES)), trace=trace)
    res = LAST_RESULT.results

    full = np.empty((B, SL, NOUT), dtype=np.float32)
    for c in range(NCORES):
        o = res[c]["out"]
        for s in range(BPC):
            full[int(order[s * NCORES + c])] = o[s]
    return full
